# revision 13
# baseline (speedup 1.0000x reference)
"""DCRNN baseline (GraphGRU over road graph) as a Bass/Tile kernel on 8 TRN2 cores.

Model (per reference):
    per step l:  Ah = A @ h            [B,N,H]   (graph aggregation)
                 gates = xt@W_x + Ah@W_h + biases
                 z = sig, r = sig, n = tanh(nx + r*nh)
                 h = (1-z)*n + z*h
    head:        y = (h @ W_head + b_head).transpose(0,2,1)   [B,HOR,N]

Sharding: data-parallel over batch B=32 across 8 cores (B_loc=4). A and all
weights replicated; no collectives.

Per-core layouts (free dim "(b,i)" is b-major, 4*1024 = 4096 cols):
    Hmat  [j, (b,h)]  : 8 j-tiles [128, 512]  — lhsT for the A@h matmul
    A_T   [j, i]      : 8 j-tiles [128, 1024] — rhs   for the A@h matmul
    AhT   [h, (b,i)]  : [128, 4096]           — MM1 out, rhs for gate matmul
    gates [g, (b,i)]  : PSUM tiles per 512-chunk; biases are per-partition
    Hrow  [h, (b,i)]  : [128, 4096]           — hidden state for elementwise
    per-step transpose Hrow -> Hmat via PE transpose (identity matmul)

All data fp32; matmuls run as float32r (full PE rate at free>=256).
"""

import numpy as np

B, L, N, F, H, HOR = 32, 12, 1024, 2, 128, 12
NCORES = 8
BLOC = B // NCORES          # 4 batch elements per core
NB = BLOC * N               # 4096 free columns "(b,i)"
JT = N // 128               # 8 j-tiles
CHUNK = 512                 # free-dim chunk for PSUM waves
NCH = NB // CHUNK           # 8 chunks
G3 = 3 * H

_COMPILED = None            # (nc, in_names) cache


def _build_program():
    import concourse.bass as bass
    import concourse.mybir as mybir
    import concourse.tile as tile
    from concourse import bacc

    f32 = mybir.dt.float32
    f32r = mybir.dt.float32r
    AF = mybir.ActivationFunctionType
    ALU = mybir.AluOpType

    nc = bacc.Bacc("TRN2", target_bir_lowering=False, debug=False,
                   num_devices=NCORES)

    xt = nc.dram_tensor("xt", [L, F, NB], f32r, kind="ExternalInput").ap()
    a_t = nc.dram_tensor("a_t", [N, N], f32r, kind="ExternalInput").ap()
    w_h = nc.dram_tensor("w_h", [H, G3], f32r, kind="ExternalInput").ap()
    w_x = nc.dram_tensor("w_x", [F, G3], f32r, kind="ExternalInput").ap()
    bz_d = nc.dram_tensor("bz", [H, 1], f32, kind="ExternalInput").ap()
    br_d = nc.dram_tensor("br", [H, 1], f32, kind="ExternalInput").ap()
    bhn_d = nc.dram_tensor("bhn", [H, 1], f32, kind="ExternalInput").ap()
    bxn_d = nc.dram_tensor("bxn", [H, 1], f32, kind="ExternalInput").ap()
    w_head = nc.dram_tensor("w_head", [H, HOR], f32r, kind="ExternalInput").ap()
    b_head = nc.dram_tensor("b_head", [HOR, 1], f32, kind="ExternalInput").ap()
    y = nc.dram_tensor("y", [BLOC, HOR, N], f32, kind="ExternalOutput").ap()

    with tile.TileContext(nc) as tc:
        with (
            tc.tile_pool(name="singles", bufs=1) as singles,
            tc.tile_pool(name="state", bufs=2) as state,
            tc.tile_pool(name="xtp", bufs=3) as xtp,
            tc.tile_pool(name="yout", bufs=2) as yout,
            tc.tile_pool(name="elw", bufs=2) as elw,
            tc.tile_pool(name="ps_ah", bufs=2, space="PSUM") as ps_ah,
            tc.tile_pool(name="ps_z", bufs=1, space="PSUM") as ps_z,
            tc.tile_pool(name="ps_r", bufs=1, space="PSUM") as ps_r,
            tc.tile_pool(name="ps_nh", bufs=1, space="PSUM") as ps_nh,
            tc.tile_pool(name="ps_nx", bufs=1, space="PSUM") as ps_nx,
            tc.tile_pool(name="ps_tr", bufs=2, space="PSUM") as ps_tr,
        ):
            # ---- resident constants ----
            at_sb = singles.tile([128, JT, N], f32r)
            nc.sync.dma_start(
                out=at_sb, in_=a_t.rearrange("(t p) i -> p t i", p=128))
            wh_sb = singles.tile([H, G3], f32r)
            nc.sync.dma_start(out=wh_sb, in_=w_h)
            wx_sb = singles.tile([F, G3], f32r)
            nc.sync.dma_start(out=wx_sb, in_=w_x)
            bz_sb = singles.tile([H, 1], f32)
            nc.sync.dma_start(out=bz_sb, in_=bz_d)
            br_sb = singles.tile([H, 1], f32)
            nc.sync.dma_start(out=br_sb, in_=br_d)
            bhn_sb = singles.tile([H, 1], f32)
            nc.sync.dma_start(out=bhn_sb, in_=bhn_d)
            bxn_sb = singles.tile([H, 1], f32)
            nc.sync.dma_start(out=bxn_sb, in_=bxn_d)
            whead_sb = singles.tile([H, HOR], f32r)
            nc.sync.dma_start(out=whead_sb, in_=w_head)
            bhead_sb = singles.tile([HOR, 1], f32)
            nc.sync.dma_start(out=bhead_sb, in_=b_head)
            ident = singles.tile([128, 128], f32)
            from concourse.masks import make_identity
            make_identity(nc, ident)

            # ---- recurrence (step 0 special-cased: h == 0) ----
            hmat = None
            hrow = None

            for l in range(L):
                first = l == 0
                aht = state.tile([128, NB], f32r, tag="aht")
                hmat_new = state.tile([128, JT, 512], f32r, tag="hmat")
                hrow_new = state.tile([128, NB], f32r, tag="hrow")

                for c in range(NCH):
                    b, half = divmod(c, N // CHUNK)
                    cs = slice(c * CHUNK, (c + 1) * CHUNK)
                    is_ = slice(half * CHUNK, (half + 1) * CHUNK)

                    xt_c = xtp.tile([F, CHUNK], f32r, tag="xt")
                    nc.sync.dma_start(out=xt_c, in_=xt[l][:, cs])

                    if not first:
                        # MM1: AhT[:, chunk] = sum_j Hmat[j,(b,h)]^T A_T[j,i]
                        pah = ps_ah.tile([128, CHUNK], f32, tag="pah")
                        for jt in range(JT):
                            nc.tensor.matmul(
                                pah,
                                lhsT=hmat[:, jt, b * H:(b + 1) * H],
                                rhs=at_sb[:, jt, is_],
                                start=(jt == 0), stop=(jt == JT - 1))
                        nc.scalar.copy(aht[:, cs], pah)

                    # MM2 + MMx: gate pre-activations in PSUM
                    rhs_ah = aht[:, cs]
                    rhs_x = xt_c
                    pz = ps_z.tile([128, CHUNK], f32, tag="pz")
                    pr = ps_r.tile([128, CHUNK], f32, tag="pr")
                    pnx = ps_nx.tile([128, CHUNK], f32, tag="pnx")
                    nc.tensor.matmul(pz, lhsT=wx_sb[:, 0:H],
                                     rhs=rhs_x, start=True, stop=first)
                    nc.tensor.matmul(pr, lhsT=wx_sb[:, H:2 * H],
                                     rhs=rhs_x, start=True, stop=first)
                    nc.tensor.matmul(pnx, lhsT=wx_sb[:, 2 * H:G3],
                                     rhs=rhs_x, start=True, stop=True)
                    if not first:
                        pnh = ps_nh.tile([128, CHUNK], f32, tag="pnh")
                        nc.tensor.matmul(pz, lhsT=wh_sb[:, 0:H],
                                         rhs=rhs_ah, start=False, stop=True)
                        nc.tensor.matmul(pr, lhsT=wh_sb[:, H:2 * H],
                                         rhs=rhs_ah, start=False, stop=True)
                        nc.tensor.matmul(pnh, lhsT=wh_sb[:, 2 * H:G3],
                                         rhs=rhs_ah, start=True, stop=True)

                    # gates + state update
                    z = elw.tile([128, CHUNK], f32, tag="z")
                    r = elw.tile([128, CHUNK], f32, tag="r")
                    tq = elw.tile([128, CHUNK], f32, tag="tq")
                    u = elw.tile([128, CHUNK], f32, tag="u")
                    n = elw.tile([128, CHUNK], f32, tag="n")
                    nc.scalar.activation(z, pz, AF.Sigmoid, bias=bz_sb)
                    nc.scalar.activation(r, pr, AF.Sigmoid, bias=br_sb)
                    if first:
                        # tq = bhn * r      (nh == 0)
                        nc.vector.tensor_scalar_mul(tq, r, bhn_sb)
                    else:
                        # tq = (nh + bhn) * r
                        nc.vector.scalar_tensor_tensor(
                            tq, in0=pnh, scalar=bhn_sb, in1=r,
                            op0=ALU.add, op1=ALU.mult)
                    nc.vector.tensor_add(u, tq, pnx)
                    nc.scalar.activation(n, u, AF.Tanh, bias=bxn_sb)
                    if first:
                        # h_new = (1 - z) * n
                        e = elw.tile([128, CHUNK], f32, tag="e")
                        nc.vector.tensor_mul(e, z, n)
                        nc.vector.tensor_sub(hrow_new[:, cs], n, e)
                    else:
                        d = elw.tile([128, CHUNK], f32, tag="d")
                        e = elw.tile([128, CHUNK], f32, tag="e")
                        nc.vector.tensor_sub(d, hrow[:, cs], n)
                        nc.vector.tensor_mul(e, z, d)
                        nc.vector.tensor_add(hrow_new[:, cs], n, e)

                    # transpose chunk of h_new back to Hmat layout
                    ptr = ps_tr.tile([128, CHUNK], f32, tag="ptr")
                    for q in range(CHUNK // 128):
                        nc.tensor.transpose(
                            ptr[:, q * 128:(q + 1) * 128],
                            hrow_new[:, c * CHUNK + q * 128:
                                     c * CHUNK + (q + 1) * 128].bitcast(f32),
                            ident)
                    nq = CHUNK // 128
                    nc.vector.tensor_copy(
                        hmat_new[:, half * nq:(half + 1) * nq,
                                 b * H:(b + 1) * H],
                        ptr.rearrange("p (q x) -> p q x", q=nq))

                hmat = hmat_new
                hrow = hrow_new

            # ---- head ----  (PSUM reused from the ps_ah pool)
            if True:
                for c in range(NCH):
                    b, half = divmod(c, N // CHUNK)
                    cs = slice(c * CHUNK, (c + 1) * CHUNK)
                    is_ = slice(half * CHUNK, (half + 1) * CHUNK)
                    ph = ps_ah.tile([HOR, CHUNK], f32, tag="pah")
                    nc.tensor.matmul(ph, lhsT=whead_sb,
                                     rhs=hrow[:, cs],
                                     start=True, stop=True)
                    yc = yout.tile([HOR, CHUNK], f32, tag="yc")
                    nc.scalar.activation(yc, ph, AF.Identity, bias=bhead_sb)
                    nc.sync.dma_start(out=y[b][:, is_], in_=yc)

    nc.compile()
    return nc


def _prep_inputs(x, A, W_x, b_x, W_h, b_h, W_head, b_head):
    f = np.float32
    A_T = np.ascontiguousarray(A.T, dtype=f)
    bsum = (b_x + b_h).astype(f)
    common = {
        "a_t": A_T,
        "w_h": np.ascontiguousarray(W_h, dtype=f),
        "w_x": np.ascontiguousarray(W_x, dtype=f),
        "bz": bsum[0:H].reshape(H, 1).copy(),
        "br": bsum[H:2 * H].reshape(H, 1).copy(),
        "bhn": np.ascontiguousarray(b_h[2 * H:G3], dtype=f).reshape(H, 1).copy(),
        "bxn": np.ascontiguousarray(b_x[2 * H:G3], dtype=f).reshape(H, 1).copy(),
        "w_head": np.ascontiguousarray(W_head, dtype=f),
        "b_head": np.ascontiguousarray(b_head, dtype=f).reshape(HOR, 1).copy(),
    }
    in_maps = []
    for i in range(NCORES):
        xs = x[i * BLOC:(i + 1) * BLOC]          # [BLOC, L, N, F]
        xt_i = np.ascontiguousarray(
            xs.transpose(1, 3, 0, 2).reshape(L, F, NB), dtype=f)
        in_maps.append({"xt": xt_i, **common})
    return in_maps


def kernel(x, A, W_x, b_x, W_h, b_h, W_head, b_head, _trace=False, _tmpdir=None):
    global _COMPILED
    from concourse.bass_utils import run_bass_kernel_spmd

    if _COMPILED is None:
        _COMPILED = _build_program()
    nc = _COMPILED

    in_maps = _prep_inputs(np.asarray(x), np.asarray(A), np.asarray(W_x),
                           np.asarray(b_x), np.asarray(W_h), np.asarray(b_h),
                           np.asarray(W_head), np.asarray(b_head))
    kw = {}
    if _trace:
        from concourse import bass_utils as _bu
        _bu.upload_artifacts = lambda tmpdir: tmpdir
        kw = dict(trace=True, tmpdir=_tmpdir)
    res = run_bass_kernel_spmd(nc, in_maps, list(range(NCORES)), **kw)

    out = np.empty((B, HOR, N), dtype=np.float32)
    for i in range(NCORES):
        out[i * BLOC:(i + 1) * BLOC] = res.results[i]["y"]
    if _trace:
        return out, res
    return out
